# revision 1
# baseline (speedup 1.0000x reference)
"""ContraFace loss kernel for 8 TRN2 NeuronCores.

Strategy: row-shard the [B, B] cosine matrix across 8 cores (B/8 = 1024 rows
per core), f2 replicated. Each core computes, for its 1024 rows:
  - sumexp[i] = sum_j exp(S * rn1_i * Vz[i, j])   (Vz = masked raw dots)
  - mx[i]     = max_j Vz[i, j]                    (masked raw dots, >= 0)
  - ps[i]     = f1_i . f2_i (own-row dot, for the positive logit)
where Vz[i, j] = (label_j != label_i) * (f1_i . f2n_j), f2n = L2-normalized f2.
The host then does the tiny O(B) combine: EMA margin m from (pos - neg), and
the cross-entropy mean, in float64.

Device pipeline per core:
  - all ACT work stays in one activation-table set (Square/Exp/Copy),
    avoiding ~1.3us table reloads; rsqrt is Newton-Raphson on DVE
  - f2 normalize (DVE per-partition scale) + transpose on TensorE (fp32r),
    software-pipelined in 8 half-panels against the main loop
  - main matmuls in float32r (full PE rate, ~1.4e-4 input rounding)
  - fused DVE scalar_tensor_tensor: (labc != labr) * psum in one pass
  - row max: DVE reduce_max; ACT Exp with per-partition scale S*rn1 and
    accum_out row-sum
"""

import sys
import os

sys.path.insert(0, "/opt/trn_rl_repo")

import numpy as np
from contextlib import ExitStack

from concourse import bass, bacc, tile
from concourse.bass_utils import run_bass_kernel_spmd
import concourse.mybir as mybir

dt = mybir.dt
Alu = mybir.AluOpType
Act = mybir.ActivationFunctionType

B, D = 8192, 512
NCORES = 8
BS = B // NCORES          # 1024 rows per core
MT = BS // 128            # 8 M-tiles per core
KC = D // 128             # 4 contraction chunks
NPANEL = 4                # f2 column panels
PW = B // NPANEL          # 2048 panel width
GW = 1024                 # group width (PSUM tile free size)
GP = PW // GW             # 2 groups per panel
NG = B // GW              # 8 groups per M-tile row
S = 64.0
EMA = 0.99

_prog_cache = {}


def _build_program():
    nc = bacc.Bacc(None)

    f1t_d = nc.declare_dram_parameter("f1t", [D, BS], dt.float32r, isOutput=False)
    f1n_d = nc.declare_dram_parameter("f1n", [BS, D], dt.float32, isOutput=False)
    f2f_d = nc.declare_dram_parameter("f2f", [B, D], dt.float32, isOutput=False)
    f2s_d = nc.declare_dram_parameter("f2s", [BS, D], dt.float32, isOutput=False)
    labc_d = nc.declare_dram_parameter("labc", [128, B], dt.uint16, isOutput=False)
    labr_d = nc.declare_dram_parameter("labr", [128, MT], dt.float32, isOutput=False)
    idn_d = nc.declare_dram_parameter("idn", [128, 128], dt.float32r, isOutput=False)

    mx_d = nc.declare_dram_parameter("mx", [128, MT * NG], dt.float32, isOutput=True)
    se_d = nc.declare_dram_parameter("se", [128, MT * NG], dt.float32, isOutput=True)
    ps_d = nc.declare_dram_parameter("ps", [128, MT], dt.float32, isOutput=True)
    rn1_d = nc.declare_dram_parameter("rn1", [128, MT], dt.float32, isOutput=True)
    rn2s_d = nc.declare_dram_parameter("rn2s", [128, MT], dt.float32, isOutput=True)

    f1n_v = f1n_d[:].rearrange("(m p) d -> p m d", p=128)
    f2s_v = f2s_d[:].rearrange("(m p) d -> p m d", p=128)
    f2f_v = f2f_d[:].rearrange("(t p) d -> p t d", p=128)
    f1t_v = f1t_d[:].rearrange("(c p) i -> p c i", p=128)

    with tile.TileContext(nc) as tc, ExitStack() as ctx:
        cst = ctx.enter_context(tc.tile_pool(name="cst", bufs=1))
        strm = ctx.enter_context(tc.tile_pool(name="strm", bufs=2))
        big = ctx.enter_context(tc.tile_pool(name="big", bufs=1))
        pan = ctx.enter_context(tc.tile_pool(name="pan", bufs=4))
        vzp = ctx.enter_context(tc.tile_pool(name="vzp", bufs=3))
        exq = ctx.enter_context(tc.tile_pool(name="exq", bufs=3))
        hvp = ctx.enter_context(tc.tile_pool(name="hvp", bufs=2))
        psm = ctx.enter_context(
            tc.tile_pool(name="psm", bufs=3, space=bass.MemorySpace.PSUM)
        )
        pst = ctx.enter_context(
            tc.tile_pool(name="pst", bufs=2, space=bass.MemorySpace.PSUM)
        )

        idn = cst.tile([128, 128], dt.float32r)
        labc = cst.tile([128, B], dt.uint16)
        labr = cst.tile([128, MT], dt.float32)
        nc.sync.dma_start(idn[:], idn_d[:])

        stats = cst.tile([128, MT * NG], dt.float32, tag="stats")
        sums = cst.tile([128, MT * NG], dt.float32, tag="sums")
        ps_t = cst.tile([128, MT], dt.float32, tag="ps")
        ssq1 = cst.tile([128, MT], dt.float32, tag="ssq1")
        ssq2s = cst.tile([128, MT], dt.float32, tag="ssq2s")
        rn1 = cst.tile([128, MT], dt.float32, tag="rn1")
        rn2s = cst.tile([128, MT], dt.float32, tag="rn2s")
        srn1 = cst.tile([128, MT], dt.float32, tag="srn1")
        tnrm = cst.tile([128, MT], dt.float32, tag="tnrm")
        ssq2 = cst.tile([128, B // 128], dt.float32, tag="ssq2")
        rn2m = cst.tile([128, B // 128], dt.float32, tag="rn2m")
        tnr2 = cst.tile([128, 16], dt.float32, tag="tnr2")

        f1t = big.tile([128, KC, BS], dt.float32r, tag="f1t")

        # rsqrt via Newton-Raphson on DVE only (no ACT table switches).
        # Constant seed ~ rsqrt(D): valid for L2^2 of D-dim unit-variance
        # gaussian rows (ssq in [~350, ~700]); 5 iterations -> fp32 exact.
        def nr_rsqrt(dst, ssq_ap, w):
            y2 = cst.tile([128, 16], dt.float32, tag="nr_y2")
            tt = cst.tile([128, 16], dt.float32, tag="nr_t")
            nc.vector.memset(dst, float(D) ** -0.5)
            for _ in range(4):
                nc.vector.tensor_tensor(out=y2[:, :w], in0=dst, in1=dst, op=Alu.mult)
                nc.vector.tensor_tensor(out=tt[:, :w], in0=ssq_ap, in1=y2[:, :w], op=Alu.mult)
                nc.vector.tensor_scalar(out=tt[:, :w], in0=tt[:, :w], scalar1=-0.5,
                                        scalar2=1.5, op0=Alu.mult, op1=Alu.add)
                nc.vector.tensor_tensor(out=dst, in0=dst, in1=tt[:, :w], op=Alu.mult)


        # ---- Steps B+C: software-pipelined half-panels -----------------
        # 8 halves of 1024 f2-rows each; half h feeds main groups (m, g=h).
        HN = NG  # 8
        f2hs = {}

        def emit_prep_half(h):
            qds = []
            for q in range(2):
                qd = strm.tile([128, 4, D], dt.float32, tag="sa")
                base = h * 8 + q * 4
                nc.sync.dma_start(qd[:], f2f_v[:, base : base + 4, :])
                for t4 in range(4):
                    gt = base + t4
                    sqs = strm.tile([128, D], dt.float32, tag="sq")
                    nc.scalar.activation(
                        sqs[:], qd[:, t4, :], Act.Square,
                        accum_out=ssq2[:, gt : gt + 1],
                    )
                qds.append(qd)
            nr_rsqrt(rn2m[:, h * 8 : h * 8 + 8], ssq2[:, h * 8 : h * 8 + 8], 8)
            return qds

        def emit_prep_tile(h, t, qds):
            gt = h * 8 + t
            f2h = f2hs[h]
            ftn = strm.tile([128, D], dt.float32r, tag="sc")
            nc.vector.tensor_scalar(
                out=ftn[:], in0=qds[t // 4][:, t % 4, :],
                scalar1=rn2m[:, gt : gt + 1],
                scalar2=None, op0=Alu.mult,
            )
            pt = pst.tile([128, 512], dt.float32r, tag="pt")
            for c in range(KC):
                nc.tensor.transpose(
                    pt[:, c * 128 : (c + 1) * 128],
                    ftn[:, c * 128 : (c + 1) * 128],
                    idn[:],
                )
            nc.scalar.copy(
                f2h[:, :, t * 128 : (t + 1) * 128],
                pt[:].rearrange("p (c i) -> p c i", c=KC),
            )

        def emit_main_group(h, m):
            g = h
            f2h = f2hs[h]
            acc = psm.tile([128, GW], dt.float32, tag="acc")
            for sidx in range(GW // 512):
                for c in range(KC):
                    nc.tensor.matmul(
                        acc[:, sidx * 512 : (sidx + 1) * 512],
                        f1t[:, c, m * 128 : (m + 1) * 128],
                        f2h[:, c, sidx * 512 : (sidx + 1) * 512],
                        start=(c == 0),
                        stop=(c == KC - 1),
                    )
            vz = vzp.tile([128, GW], dt.float32, tag="vz")
            nc.vector.scalar_tensor_tensor(
                out=vz[:],
                in0=labc[:, g * GW : (g + 1) * GW],
                scalar=labr[:, m : m + 1],
                in1=acc[:],
                op0=Alu.not_equal,
                op1=Alu.mult,
            )
            nc.vector.tensor_reduce(
                out=stats[:, m * NG + g : m * NG + g + 1],
                in_=vz[:],
                axis=mybir.AxisListType.X,
                op=Alu.max,
            )
            ex = exq.tile([128, GW], dt.bfloat16, tag="ex")
            nc.scalar.activation(
                ex[:],
                vz[:],
                Act.Exp,
                bias=0.0,
                scale=srn1[:, m : m + 1],
                accum_out=sums[:, m * NG + g : m * NG + g + 1],
            )

        # prologue: prep halves 0 and 1
        f2h_new = pan.tile([128, KC, GW], dt.float32r, tag="f2p")
        f2hs[0] = f2h_new
        qds0 = emit_prep_half(0)
        for t in range(8):
            emit_prep_tile(0, t, qds0)

        nc.sync.dma_start(f1t[:], f1t_v)
        nc.sync.dma_start(labc[:], labc_d[:])
        nc.sync.dma_start(labr[:], labr_d[:])

        # ---- Step A: f1 norms, own-f2 norms, positive dots -------------
        abt = cst.tile([128, 2, MT, D], dt.float32, tag="abt")
        nc.gpsimd.dma_start(abt[:, 0, :, :], f1n_v)
        nc.gpsimd.dma_start(abt[:, 1, :, :], f2s_v)
        for m in range(MT):
            c = strm.tile([128, D], dt.float32, tag="sc")
            nc.vector.scalar_tensor_tensor(
                out=c[:], in0=abt[:, 0, m, :], scalar=1.0, in1=abt[:, 1, m, :],
                op0=Alu.mult, op1=Alu.mult, accum_out=ps_t[:, m : m + 1],
            )
            nc.scalar.activation(abt[:, 0, m, :], abt[:, 0, m, :], Act.Square,
                                 accum_out=ssq1[:, m : m + 1])
            nc.scalar.activation(abt[:, 1, m, :], abt[:, 1, m, :], Act.Square,
                                 accum_out=ssq2s[:, m : m + 1])

        nr_rsqrt(rn1[:], ssq1[:], MT)
        nr_rsqrt(rn2s[:], ssq2s[:], MT)
        nc.vector.tensor_scalar_mul(srn1[:], rn1[:], S)


        f2h_new = pan.tile([128, KC, GW], dt.float32r, tag="f2p")
        f2hs[1] = f2h_new
        qds0 = emit_prep_half(1)
        for t in range(8):
            emit_prep_tile(1, t, qds0)

        for h in range(HN):
            if h + 2 < HN:
                f2h_new = pan.tile([128, KC, GW], dt.float32r, tag="f2p")
                f2hs[h + 2] = f2h_new
            qds = None
            for m in range(MT):
                emit_main_group(h, m)
                if h + 2 < HN:
                    if m == 0:
                        qds = emit_prep_half(h + 2)
                    emit_prep_tile(h + 2, m, qds)

        nc.sync.dma_start(mx_d[:], stats[:])
        nc.sync.dma_start(se_d[:], sums[:])
        nc.sync.dma_start(ps_d[:], ps_t[:])
        nc.sync.dma_start(rn1_d[:], rn1[:])
        nc.sync.dma_start(rn2s_d[:], rn2s[:])

    if not nc.is_finalized():
        nc.finalize()
    return nc


def _get_program():
    if "nc" not in _prog_cache:
        _prog_cache["nc"] = _build_program()
    return _prog_cache["nc"]


def kernel(feature1, feature2, label, _want_results=False, _trace=False):
    f1 = np.ascontiguousarray(np.asarray(feature1, dtype=np.float32))
    f2 = np.ascontiguousarray(np.asarray(feature2, dtype=np.float32))
    lab = np.asarray(label)
    lab_u16 = lab.astype(np.uint16)
    labc = np.ascontiguousarray(np.broadcast_to(lab_u16[None, :], (128, B)))
    idn = np.eye(128, dtype=np.float32)

    in_maps = []
    for c in range(NCORES):
        sl = slice(c * BS, (c + 1) * BS)
        f1s = f1[sl]
        in_maps.append(
            dict(
                f1t=np.ascontiguousarray(f1s.T),
                f1n=f1s,
                f2f=f2,
                f2s=np.ascontiguousarray(f2[sl]),
                labc=labc,
                labr=np.ascontiguousarray(
                    lab[sl].reshape(MT, 128).T.astype(np.float32)
                ),
                idn=idn,
            )
        )

    nc = _get_program()
    kw = {}
    if _trace:
        kw = dict(trace=True)
    out = run_bass_kernel_spmd(nc, in_maps, list(range(NCORES)), **kw)
    res = out.results

    pos = np.empty(B, dtype=np.float64)
    neg = np.empty(B, dtype=np.float64)
    sumoff = np.empty(B, dtype=np.float64)
    for c in range(NCORES):
        r = res[c]
        sl = slice(c * BS, (c + 1) * BS)
        rn1 = r["rn1"].astype(np.float64)      # [128, MT]
        rn2s = r["rn2s"].astype(np.float64)
        ps = r["ps"].astype(np.float64)
        mx = r["mx"].astype(np.float64).reshape(128, MT, NG)
        se = r["se"].astype(np.float64).reshape(128, MT, NG)
        p = np.clip(ps * rn1 * rn2s, -1.0, 1.0)           # [128, MT]
        n = np.maximum(0.0, rn1 * mx.max(axis=2))          # [128, MT]
        so = se.sum(axis=2) - 1.0                          # [128, MT]
        pos[sl] = p.T.reshape(BS)
        neg[sl] = n.T.reshape(BS)
        sumoff[sl] = so.T.reshape(BS)

    m = EMA * np.mean(pos - neg)
    z = S * (pos - m)
    loss = np.mean(np.log(sumoff + np.exp(z)) - z)
    out_val = np.float32(loss)
    if _want_results:
        return out_val, out
    return out_val



# revision 5
# speedup vs baseline: 1.7380x; 1.7380x over previous
"""ContraFace loss kernel for 8 TRN2 NeuronCores.

Strategy: row-shard the [B, B] cosine matrix across 8 cores (B/8 = 1024 rows
per core). Host supplies layout-prepped shards:
  - f2nt: L2-normalized f2, transposed to [D, B] (bf16) - the matmul moving
    operand, shared by all cores
  - f1t:  raw f1 shard transposed [D, BS] (bf16) - stationary operand
  - maskf: [128, MT, B] uint8 0/1 mask, 0 where label_col == label_row
    (zeroes same-label entries incl. the diagonal, matching the reference's
    cos=0 substitution)
  - f1n/f2ns: row-major shards (bf16) for the positive-pair dots and f1 norms

Device pipeline per core, per (m, g) tile of the [1024, 8192] block:
  - PE: 16 bf16 matmuls accumulate a [128, 2048] PSUM tile (1 cycle/row)
  - DVE tensor_tensor: vz = psum * mask (bf16 out)
  - DVE tensor_scalar on vz (4x bf16 mode) with accum_out op1=max -> row max
  - ACT Exp with per-partition scale S*rn1 and accum_out -> row sumexp
rn1 (f1 row rsqrt-norms) comes from ACT Square + Newton-Raphson on DVE.
Host does the tiny O(B) combine in float64: EMA margin m from (pos - neg),
cross-entropy mean.
"""

import sys
import os

sys.path.insert(0, "/opt/trn_rl_repo")

import numpy as np
from contextlib import ExitStack

import ml_dtypes

from concourse import bass, bacc, tile
from concourse.bass_utils import run_bass_kernel_spmd
import concourse.mybir as mybir

dt = mybir.dt
Alu = mybir.AluOpType
Act = mybir.ActivationFunctionType

B, D = 8192, 512
NCORES = 8
BS = B // NCORES          # 1024 rows per core
MT = BS // 128            # 8 M-tiles per core
KC = D // 128             # 4 contraction chunks
GW = 2048                 # column group width (PSUM tile free size, 4 banks)
NG = B // GW              # 4 column groups
S = 64.0
EMA = 0.99

_prog_cache = {}


def _build_program():
    nc = bacc.Bacc(None)

    f1t_d = nc.declare_dram_parameter("f1t", [D, BS], dt.bfloat16, isOutput=False)
    f2nt_d = nc.declare_dram_parameter("f2nt", [D, B], dt.bfloat16, isOutput=False)
    mask_d = nc.declare_dram_parameter("maskf", [128, MT, B], dt.uint8, isOutput=False)
    f1n_d = nc.declare_dram_parameter("f1n", [BS, D], dt.bfloat16, isOutput=False)
    f2ns_d = nc.declare_dram_parameter("f2ns", [BS, D], dt.bfloat16, isOutput=False)

    mx_d = nc.declare_dram_parameter("mx", [128, MT * NG], dt.float32, isOutput=True)
    se_d = nc.declare_dram_parameter("se", [128, MT * NG], dt.float32, isOutput=True)
    ps_d = nc.declare_dram_parameter("ps", [128, MT], dt.float32, isOutput=True)
    rn1_d = nc.declare_dram_parameter("rn1", [128, MT], dt.float32, isOutput=True)

    f1t_v = f1t_d[:].rearrange("(c p) i -> p c i", p=128)
    f2nt_v = f2nt_d[:].rearrange("(c p) j -> p c j", p=128)
    f1n_v = f1n_d[:].rearrange("(m p) d -> p m d", p=128)
    f2ns_v = f2ns_d[:].rearrange("(m p) d -> p m d", p=128)

    with tile.TileContext(nc) as tc, ExitStack() as ctx:
        cst = ctx.enter_context(tc.tile_pool(name="cst", bufs=1))
        pan = ctx.enter_context(tc.tile_pool(name="pan", bufs=NG))
        mkp = ctx.enter_context(tc.tile_pool(name="mkp", bufs=NG))
        vzp = ctx.enter_context(tc.tile_pool(name="vzp", bufs=3))
        exq = ctx.enter_context(tc.tile_pool(name="exq", bufs=2))
        dmp = ctx.enter_context(tc.tile_pool(name="dmp", bufs=2))
        psm = ctx.enter_context(
            tc.tile_pool(name="psm", bufs=2, space=bass.MemorySpace.PSUM)
        )

        stats = cst.tile([128, MT * NG], dt.float32, tag="stats")
        sums = cst.tile([128, MT * NG], dt.float32, tag="sums")
        ps_t = cst.tile([128, MT], dt.float32, tag="ps")
        ssq1 = cst.tile([128, MT], dt.float32, tag="ssq1")
        rn1 = cst.tile([128, MT], dt.float32, tag="rn1")
        srn1 = cst.tile([128, MT], dt.float32, tag="srn1")

        f1t_sb = cst.tile([128, KC, BS], dt.bfloat16, tag="f1t")
        abt = cst.tile([128, 2, MT, D], dt.bfloat16, tag="abt")

        f2p = []
        mk = []
        for g in range(NG):
            f2p.append(pan.tile([128, KC, GW], dt.bfloat16, tag="f2p", name=f"f2p{g}"))
            mk.append(mkp.tile([128, MT, GW], dt.uint8, tag="mk", name=f"mk{g}"))

        # DMA priority order (transfers serialize on the shared DMA device):
        # what the PE needs first goes first.
        nc.sync.dma_start(f1t_sb[:], f1t_v)
        nc.sync.dma_start(f2p[0][:], f2nt_v[:, :, 0:GW])
        nc.sync.dma_start(mk[0][:], mask_d[:, :, 0:GW])
        nc.sync.dma_start(abt[:, 0, :, :], f1n_v)
        nc.sync.dma_start(abt[:, 1, :, :], f2ns_v)
        for g in range(1, NG):
            nc.sync.dma_start(f2p[g][:], f2nt_v[:, :, g * GW : (g + 1) * GW])
            nc.sync.dma_start(mk[g][:], mask_d[:, :, g * GW : (g + 1) * GW])

        # rsqrt via Newton-Raphson on DVE only (no ACT table switches).
        # Constant seed ~ rsqrt(D): valid for L2^2 of D-dim unit-variance
        # gaussian rows (ssq in [~350, ~700]); 4 iterations -> fp32 exact.
        def nr_rsqrt(dst, ssq_ap, w):
            y2 = cst.tile([128, 16], dt.float32, tag="nr_y2")
            tt = cst.tile([128, 16], dt.float32, tag="nr_t")
            nc.vector.memset(dst, float(D) ** -0.5)
            for _ in range(4):
                nc.vector.tensor_tensor(out=y2[:, :w], in0=dst, in1=dst, op=Alu.mult)
                nc.vector.tensor_tensor(out=tt[:, :w], in0=ssq_ap, in1=y2[:, :w], op=Alu.mult)
                nc.vector.tensor_scalar(out=tt[:, :w], in0=tt[:, :w], scalar1=-0.5,
                                        scalar2=1.5, op0=Alu.mult, op1=Alu.add)
                nc.vector.tensor_tensor(out=dst, in0=dst, in1=tt[:, :w], op=Alu.mult)

        # ---- Step A: f1 norms + positive dots (overlaps main-loop prologue)
        dots = cst.tile([128, D], dt.float32, tag="dots")
        for m in range(MT):
            nc.vector.scalar_tensor_tensor(
                out=dots[:], in0=abt[:, 0, m, :], scalar=1.0, in1=abt[:, 1, m, :],
                op0=Alu.mult, op1=Alu.mult, accum_out=ps_t[:, m : m + 1],
            )
            nc.scalar.activation(abt[:, 0, m, :], abt[:, 0, m, :], Act.Square,
                                 accum_out=ssq1[:, m : m + 1])
        nr_rsqrt(rn1[:], ssq1[:], MT)
        nc.vector.tensor_scalar_mul(srn1[:], rn1[:], S)

        # ---- Main loop: matmul -> masked max (DVE ttr) -> exp+sum (ACT)
        for g in range(NG):
            for m in range(MT):
                acc = psm.tile([128, GW], dt.float32, tag="acc")
                for s in range(GW // 512):
                    for c in range(KC):
                        nc.tensor.matmul(
                            acc[:, s * 512 : (s + 1) * 512],
                            f1t_sb[:, c, m * 128 : (m + 1) * 128],
                            f2p[g][:, c, s * 512 : (s + 1) * 512],
                            start=(c == 0),
                            stop=(c == KC - 1),
                        )
                vz = vzp.tile([128, GW], dt.bfloat16, tag="vz")
                nc.vector.tensor_tensor(
                    out=vz[:], in0=acc[:], in1=mk[g][:, m, :], op=Alu.mult
                )
                dum = dmp.tile([128, GW], dt.bfloat16, tag="dum")
                nc.vector.tensor_scalar(
                    out=dum[:], in0=vz[:], scalar1=1.0, scalar2=None,
                    op0=Alu.mult, op1=Alu.max,
                    accum_out=stats[:, m * NG + g : m * NG + g + 1],
                )
                ex = exq.tile([128, GW], dt.bfloat16, tag="ex")
                nc.scalar.activation(
                    ex[:],
                    vz[:],
                    Act.Exp,
                    bias=0.0,
                    scale=srn1[:, m : m + 1],
                    accum_out=sums[:, m * NG + g : m * NG + g + 1],
                )

        nc.sync.dma_start(mx_d[:], stats[:])
        nc.sync.dma_start(se_d[:], sums[:])
        nc.sync.dma_start(ps_d[:], ps_t[:])
        nc.sync.dma_start(rn1_d[:], rn1[:])

    if not nc.is_finalized():
        nc.finalize()
    return nc


def _get_program():
    if "nc" not in _prog_cache:
        _prog_cache["nc"] = _build_program()
    return _prog_cache["nc"]


def _host_inputs(feature1, feature2, label):
    bf16 = ml_dtypes.bfloat16
    f1 = np.asarray(feature1, dtype=np.float32)
    f2 = np.asarray(feature2, dtype=np.float32)
    lab = np.asarray(label)

    f2n = f2 / np.linalg.norm(f2, axis=1, keepdims=True)
    f2nt = np.ascontiguousarray(f2n.T.astype(bf16))

    in_maps = []
    for c in range(NCORES):
        sl = slice(c * BS, (c + 1) * BS)
        f1s = f1[sl]
        same = lab[sl, None] == lab[None, :]                  # [BS, B]
        maskf = np.ascontiguousarray(
            (~same).astype(np.uint8).reshape(MT, 128, B).transpose(1, 0, 2)
        )
        in_maps.append(
            dict(
                f1t=np.ascontiguousarray(f1s.T.astype(bf16)),
                f2nt=f2nt,
                maskf=maskf,
                f1n=np.ascontiguousarray(f1s.astype(bf16)),
                f2ns=np.ascontiguousarray(f2n[sl].astype(bf16)),
            )
        )
    return in_maps


def kernel(feature1, feature2, label, _want_results=False, _trace=False):
    in_maps = _host_inputs(feature1, feature2, label)

    nc = _get_program()
    kw = {}
    if _trace:
        kw = dict(trace=True)
    out = run_bass_kernel_spmd(nc, in_maps, list(range(NCORES)), **kw)
    res = out.results

    pos = np.empty(B, dtype=np.float64)
    neg = np.empty(B, dtype=np.float64)
    sumoff = np.empty(B, dtype=np.float64)
    for c in range(NCORES):
        r = res[c]
        sl = slice(c * BS, (c + 1) * BS)
        rn1 = r["rn1"].astype(np.float64)                  # [128, MT]
        ps = r["ps"].astype(np.float64)
        mx = r["mx"].astype(np.float64).reshape(128, MT, NG)
        se = r["se"].astype(np.float64).reshape(128, MT, NG)
        p = np.clip(ps * rn1, -1.0, 1.0)                   # [128, MT]
        n = rn1 * mx.max(axis=2)                           # >= 0 by ttr init
        so = se.sum(axis=2) - 1.0
        pos[sl] = p.T.reshape(BS)
        neg[sl] = n.T.reshape(BS)
        sumoff[sl] = so.T.reshape(BS)

    m = EMA * np.mean(pos - neg)
    z = S * (pos - m)
    loss = np.mean(np.log(sumoff + np.exp(z)) - z)
    out_val = np.float32(loss)
    if _want_results:
        return out_val, out
    return out_val


# revision 6
# speedup vs baseline: 1.8817x; 1.0827x over previous
"""ContraFace loss kernel for 8 TRN2 NeuronCores.

Strategy: row-shard the [B, B] cosine matrix across 8 cores (B/8 = 1024 rows
per core). Host supplies layout-prepped shards:
  - f2nt: L2-normalized f2, transposed to [D, B] (bf16) - the matmul moving
    operand, shared by all cores
  - f1t:  raw f1 shard transposed [D, BS] (bf16) - stationary operand
  - maskf: [128, MT, B] uint8 0/1 mask, 0 where label_col == label_row
    (zeroes same-label entries incl. the diagonal, matching the reference's
    cos=0 substitution)
  - f1n/f2ns: row-major shards (bf16) for the positive-pair dots and f1 norms

Device pipeline per core, per (m, g) tile of the [1024, 8192] block:
  - PE: 16 bf16 matmuls accumulate a [128, 2048] PSUM tile (1 cycle/row)
  - DVE tensor_tensor: vz = psum * mask (bf16 out)
  - DVE tensor_scalar on vz (4x bf16 mode) with accum_out op1=max -> row max
  - ACT Exp with per-partition scale S*rn1 and accum_out -> row sumexp
DMAs are split/ordered so the PE starts ~6us in and never starves; rn1 (f1
row rsqrt-norms) comes from ACT Square into scratch + Newton-Raphson on DVE,
gated only on the f1n DMA so the first Exp isn't delayed.
Host does the tiny O(B) combine in float64: EMA margin m from (pos - neg),
cross-entropy mean.
"""

import sys
import os

sys.path.insert(0, "/opt/trn_rl_repo")

import numpy as np
from contextlib import ExitStack

import ml_dtypes

from concourse import bass, bacc, tile
from concourse.bass_utils import run_bass_kernel_spmd
import concourse.mybir as mybir

dt = mybir.dt
Alu = mybir.AluOpType
Act = mybir.ActivationFunctionType

B, D = 8192, 512
NCORES = 8
BS = B // NCORES          # 1024 rows per core
MT = BS // 128            # 8 M-tiles per core
KC = D // 128             # 4 contraction chunks
GW = 2048                 # column group width (PSUM tile free size, 4 banks)
NG = B // GW              # 4 column groups
S = 64.0
EMA = 0.99

_prog_cache = {}


def _build_program():
    nc = bacc.Bacc(None)

    f1t_d = nc.declare_dram_parameter("f1t", [D, BS], dt.bfloat16, isOutput=False)
    f2nt_d = nc.declare_dram_parameter("f2nt", [D, B], dt.bfloat16, isOutput=False)
    mask_d = nc.declare_dram_parameter("maskf", [128, MT, B], dt.uint8, isOutput=False)
    f1n_d = nc.declare_dram_parameter("f1n", [BS, D], dt.bfloat16, isOutput=False)
    f2ns_d = nc.declare_dram_parameter("f2ns", [BS, D], dt.bfloat16, isOutput=False)

    # g-major stats layout: column g*MT + m, so per-group slices are
    # contiguous and can be written back as soon as each group drains.
    mx_d = nc.declare_dram_parameter("mx", [128, NG * MT], dt.float32, isOutput=True)
    se_d = nc.declare_dram_parameter("se", [128, NG * MT], dt.float32, isOutput=True)
    ps_d = nc.declare_dram_parameter("ps", [128, MT], dt.float32, isOutput=True)
    rn1_d = nc.declare_dram_parameter("rn1", [128, MT], dt.float32, isOutput=True)

    f1t_v = f1t_d[:].rearrange("(c p) i -> p c i", p=128)
    f2nt_v = f2nt_d[:].rearrange("(c p) j -> p c j", p=128)
    f1n_v = f1n_d[:].rearrange("(m p) d -> p m d", p=128)
    f2ns_v = f2ns_d[:].rearrange("(m p) d -> p m d", p=128)

    with tile.TileContext(nc) as tc, ExitStack() as ctx:
        cst = ctx.enter_context(tc.tile_pool(name="cst", bufs=1))
        pan = ctx.enter_context(tc.tile_pool(name="pan", bufs=NG))
        mkp = ctx.enter_context(tc.tile_pool(name="mkp", bufs=NG))
        vzp = ctx.enter_context(tc.tile_pool(name="vzp", bufs=3))
        exq = ctx.enter_context(tc.tile_pool(name="exq", bufs=2))
        dmp = ctx.enter_context(tc.tile_pool(name="dmp", bufs=2))
        sqp = ctx.enter_context(tc.tile_pool(name="sqp", bufs=2))
        psm = ctx.enter_context(
            tc.tile_pool(name="psm", bufs=2, space=bass.MemorySpace.PSUM)
        )

        stats = cst.tile([128, NG * MT], dt.float32, tag="stats")
        sums = cst.tile([128, NG * MT], dt.float32, tag="sums")
        ps_t = cst.tile([128, MT], dt.float32, tag="ps")
        ssq1 = cst.tile([128, MT], dt.float32, tag="ssq1")
        rn1 = cst.tile([128, MT], dt.float32, tag="rn1")
        srn1 = cst.tile([128, MT], dt.float32, tag="srn1")

        f1t_sb = cst.tile([128, KC, BS], dt.bfloat16, tag="f1t")
        f1n_sb = cst.tile([128, MT, D], dt.bfloat16, tag="f1n")
        f2ns_sb = cst.tile([128, MT, D], dt.bfloat16, tag="f2ns")

        f2p = []
        mk = []
        for g in range(NG):
            f2p.append(pan.tile([128, KC, GW], dt.bfloat16, tag="f2p", name=f"f2p{g}"))
            mk.append(mkp.tile([128, MT, GW], dt.uint8, tag="mk", name=f"mk{g}"))

        # DMA priority order (transfers serialize on the shared DMA device):
        # the PE prologue (first f1t half + first f2 half-panel) goes first,
        # then the first tt's mask slice, the rest interleaved by need-time.
        HB = BS // 2
        nc.sync.dma_start(f1t_sb[:, :, 0:HB], f1t_v[:, :, 0:HB])
        nc.sync.dma_start(f2p[0][:, :, 0 : GW // 2], f2nt_v[:, :, 0 : GW // 2])
        nc.sync.dma_start(mk[0][:, 0:2, :], mask_d[:, 0:2, 0:GW])
        nc.sync.dma_start(f2p[0][:, :, GW // 2 : GW], f2nt_v[:, :, GW // 2 : GW])
        nc.sync.dma_start(f1n_sb[:], f1n_v)
        nc.sync.dma_start(mk[0][:, 2:MT, :], mask_d[:, 2:MT, 0:GW])
        nc.sync.dma_start(f1t_sb[:, :, HB:BS], f1t_v[:, :, HB:BS])
        nc.sync.dma_start(f2p[1][:], f2nt_v[:, :, GW : 2 * GW])
        nc.sync.dma_start(f2ns_sb[:], f2ns_v)
        nc.sync.dma_start(mk[1][:], mask_d[:, :, GW : 2 * GW])
        for g in range(2, NG):
            nc.sync.dma_start(f2p[g][:], f2nt_v[:, :, g * GW : (g + 1) * GW])
            nc.sync.dma_start(mk[g][:], mask_d[:, :, g * GW : (g + 1) * GW])

        # rsqrt via Newton-Raphson on DVE only (no ACT table switches).
        # Constant seed ~ rsqrt(D): valid for L2^2 of D-dim unit-variance
        # gaussian rows (ssq in [~350, ~700]); 4 iterations -> fp32 exact.
        def nr_rsqrt(dst, ssq_ap, w):
            y2 = cst.tile([128, 16], dt.float32, tag="nr_y2")
            tt = cst.tile([128, 16], dt.float32, tag="nr_t")
            nc.vector.memset(dst, float(D) ** -0.5)
            for _ in range(4):
                nc.vector.tensor_tensor(out=y2[:, :w], in0=dst, in1=dst, op=Alu.mult)
                nc.vector.tensor_tensor(out=tt[:, :w], in0=ssq_ap, in1=y2[:, :w], op=Alu.mult)
                nc.vector.tensor_scalar(out=tt[:, :w], in0=tt[:, :w], scalar1=-0.5,
                                        scalar2=1.5, op0=Alu.mult, op1=Alu.add)
                nc.vector.tensor_tensor(out=dst, in0=dst, in1=tt[:, :w], op=Alu.mult)

        # ---- Step A1: f1 norms (gated only on the f1n DMA; srn1 feeds the
        # first Exp). Squares go to scratch so f1n stays intact for step A2.
        for m in range(MT):
            sq = sqp.tile([128, D], dt.bfloat16, tag="sq")
            nc.scalar.activation(sq[:], f1n_sb[:, m, :], Act.Square,
                                 accum_out=ssq1[:, m : m + 1])
        nr_rsqrt(rn1[:], ssq1[:], MT)
        nc.vector.tensor_scalar_mul(srn1[:], rn1[:], S)
        nc.sync.dma_start(rn1_d[:], rn1[:])

        # ---- Step A2: positive dots (needs f2ns, arrives later)
        dots = cst.tile([128, D], dt.float32, tag="dots")
        for m in range(MT):
            nc.vector.scalar_tensor_tensor(
                out=dots[:], in0=f1n_sb[:, m, :], scalar=1.0, in1=f2ns_sb[:, m, :],
                op0=Alu.mult, op1=Alu.mult, accum_out=ps_t[:, m : m + 1],
            )
        nc.sync.dma_start(ps_d[:], ps_t[:])

        # ---- Main loop: matmul -> mask (DVE tt) -> max (DVE ts) -> exp (ACT)
        for g in range(NG):
            for m in range(MT):
                acc = psm.tile([128, GW], dt.float32, tag="acc")
                for s in range(GW // 512):
                    for c in range(KC):
                        nc.tensor.matmul(
                            acc[:, s * 512 : (s + 1) * 512],
                            f1t_sb[:, c, m * 128 : (m + 1) * 128],
                            f2p[g][:, c, s * 512 : (s + 1) * 512],
                            start=(c == 0),
                            stop=(c == KC - 1),
                        )
                vz = vzp.tile([128, GW], dt.bfloat16, tag="vz")
                nc.vector.tensor_tensor(
                    out=vz[:], in0=acc[:], in1=mk[g][:, m, :], op=Alu.mult
                )
                dum = dmp.tile([128, GW], dt.bfloat16, tag="dum")
                nc.vector.tensor_scalar(
                    out=dum[:], in0=vz[:], scalar1=1.0, scalar2=None,
                    op0=Alu.mult, op1=Alu.max,
                    accum_out=stats[:, g * MT + m : g * MT + m + 1],
                )
                ex = exq.tile([128, GW], dt.bfloat16, tag="ex")
                nc.scalar.activation(
                    ex[:],
                    vz[:],
                    Act.Exp,
                    bias=0.0,
                    scale=srn1[:, m : m + 1],
                    accum_out=sums[:, g * MT + m : g * MT + m + 1],
                )
            # write back this group's stats as soon as its chain drains
            nc.sync.dma_start(mx_d[:, g * MT : (g + 1) * MT],
                              stats[:, g * MT : (g + 1) * MT])
            nc.sync.dma_start(se_d[:, g * MT : (g + 1) * MT],
                              sums[:, g * MT : (g + 1) * MT])

    if not nc.is_finalized():
        nc.finalize()
    return nc


def _get_program():
    if "nc" not in _prog_cache:
        _prog_cache["nc"] = _build_program()
    return _prog_cache["nc"]


def _host_inputs(feature1, feature2, label):
    bf16 = ml_dtypes.bfloat16
    f1 = np.asarray(feature1, dtype=np.float32)
    f2 = np.asarray(feature2, dtype=np.float32)
    lab = np.asarray(label)

    f2n = f2 / np.linalg.norm(f2, axis=1, keepdims=True)
    f2nt = np.ascontiguousarray(f2n.T.astype(bf16))

    in_maps = []
    for c in range(NCORES):
        sl = slice(c * BS, (c + 1) * BS)
        f1s = f1[sl]
        same = lab[sl, None] == lab[None, :]                  # [BS, B]
        maskf = np.ascontiguousarray(
            (~same).astype(np.uint8).reshape(MT, 128, B).transpose(1, 0, 2)
        )
        in_maps.append(
            dict(
                f1t=np.ascontiguousarray(f1s.T.astype(bf16)),
                f2nt=f2nt,
                maskf=maskf,
                f1n=np.ascontiguousarray(f1s.astype(bf16)),
                f2ns=np.ascontiguousarray(f2n[sl].astype(bf16)),
            )
        )
    return in_maps


def kernel(feature1, feature2, label, _want_results=False, _trace=False):
    in_maps = _host_inputs(feature1, feature2, label)

    nc = _get_program()
    kw = {}
    if _trace:
        kw = dict(trace=True)
    out = run_bass_kernel_spmd(nc, in_maps, list(range(NCORES)), **kw)
    res = out.results

    pos = np.empty(B, dtype=np.float64)
    neg = np.empty(B, dtype=np.float64)
    sumoff = np.empty(B, dtype=np.float64)
    for c in range(NCORES):
        r = res[c]
        sl = slice(c * BS, (c + 1) * BS)
        rn1 = r["rn1"].astype(np.float64)                  # [128, MT]
        ps = r["ps"].astype(np.float64)
        mx = r["mx"].astype(np.float64).reshape(128, NG, MT)
        se = r["se"].astype(np.float64).reshape(128, NG, MT)
        p = np.clip(ps * rn1, -1.0, 1.0)                   # [128, MT]
        n = rn1 * mx.max(axis=1)                           # >= 0: diag masked
        so = se.sum(axis=1) - 1.0
        pos[sl] = p.T.reshape(BS)
        neg[sl] = n.T.reshape(BS)
        sumoff[sl] = so.T.reshape(BS)

    m = EMA * np.mean(pos - neg)
    z = S * (pos - m)
    loss = np.mean(np.log(sumoff + np.exp(z)) - z)
    out_val = np.float32(loss)
    if _want_results:
        return out_val, out
    return out_val


# revision 10
# speedup vs baseline: 1.9536x; 1.0383x over previous
"""ContraFace loss kernel for 8 TRN2 NeuronCores.

Strategy: row-shard the [B, B] cosine matrix across 8 cores (B/8 = 1024 rows
per core). The device does the O(B^2 D) / O(B^2) work: the full cosine
matmul, same-label masking, per-row hardest-negative max, and per-row
sum(exp(S*cos)). Host supplies layout-prepped shards:
  - f2nt: L2-normalized f2, transposed to [D, B] (bf16) - the matmul moving
    operand, shared by all cores
  - f1t:  raw f1 shard transposed [D, BS] (bf16) - stationary operand
  - maskf: [128, MT, B] uint8 0/1 mask, 0 where label_col == label_row
    (zeroes same-label entries incl. the diagonal, matching the reference's
    cos=0 substitution)
  - srn1: [128, MT] fp32 = S / ||f1_row|| per-partition Exp scale

Device pipeline per core, per (m, g) tile of the [1024, 8192] block:
  - PE: 16 bf16 matmuls accumulate a [128, 2048] PSUM tile (1 cycle/row)
  - DVE tensor_tensor: vz = psum * mask (bf16 out)
  - DVE tensor_scalar on vz (4x bf16 mode) with accum_out op1=max -> row max
  - ACT Exp with per-partition scale srn1 and accum_out -> row sumexp
DMAs are split/ordered so the PE starts ~5.5us in and never starves; the
last m-tile is split in half to shorten the end-of-kernel drain. Host does
the tiny O(B) combine in float64: positives, EMA margin m from (pos - neg),
cross-entropy mean.
"""

import sys
import os

sys.path.insert(0, "/opt/trn_rl_repo")

import numpy as np
from contextlib import ExitStack

import ml_dtypes

from concourse import bass, bacc, tile
from concourse.bass_utils import run_bass_kernel_spmd
import concourse.mybir as mybir

dt = mybir.dt
Alu = mybir.AluOpType
Act = mybir.ActivationFunctionType

B, D = 8192, 512
NCORES = 8
BS = B // NCORES          # 1024 rows per core
MT = BS // 128            # 8 M-tiles per core
KC = D // 128             # 4 contraction chunks
GW = 2048                 # column group width (PSUM tile free size, 4 banks)
NG = B // GW              # 4 column groups
S = 64.0
EMA = 0.99

_prog_cache = {}


def _build_program():
    nc = bacc.Bacc(None)

    f1t_d = nc.declare_dram_parameter("f1t", [D, BS], dt.bfloat16, isOutput=False)
    f2nt_d = nc.declare_dram_parameter("f2nt", [D, B], dt.bfloat16, isOutput=False)
    mask_d = nc.declare_dram_parameter("maskf", [128, MT, B], dt.uint8, isOutput=False)
    srn1_d = nc.declare_dram_parameter("srn1", [128, MT], dt.float32, isOutput=False)

    # g-major stats layout: column g*MT + m, so per-group slices are
    # contiguous and can be written back as soon as each group drains.
    # One extra column holds the second half of the split last tile.
    NSTAT = NG * MT + 1
    mx_d = nc.declare_dram_parameter("mx", [128, NSTAT], dt.float32, isOutput=True)
    se_d = nc.declare_dram_parameter("se", [128, NSTAT], dt.float32, isOutput=True)

    f1t_v = f1t_d[:].rearrange("(c p) i -> p c i", p=128)
    f2nt_v = f2nt_d[:].rearrange("(c p) j -> p c j", p=128)

    with tile.TileContext(nc) as tc, ExitStack() as ctx:
        cst = ctx.enter_context(tc.tile_pool(name="cst", bufs=1))
        pan = ctx.enter_context(tc.tile_pool(name="pan", bufs=NG))
        mkp = ctx.enter_context(tc.tile_pool(name="mkp", bufs=NG))
        vzp = ctx.enter_context(tc.tile_pool(name="vzp", bufs=4))
        exq = ctx.enter_context(tc.tile_pool(name="exq", bufs=3))
        dmp = ctx.enter_context(tc.tile_pool(name="dmp", bufs=2))
        psm = ctx.enter_context(
            tc.tile_pool(name="psm", bufs=2, space=bass.MemorySpace.PSUM)
        )

        stats = cst.tile([128, NG * MT + 1], dt.float32, tag="stats")
        sums = cst.tile([128, NG * MT + 1], dt.float32, tag="sums")
        srn1 = cst.tile([128, MT], dt.float32, tag="srn1")
        f1t_sb = cst.tile([128, KC, BS], dt.bfloat16, tag="f1t")

        f2p = []
        mk = []
        for g in range(NG):
            f2p.append(pan.tile([128, KC, GW], dt.bfloat16, tag="f2p", name=f"f2p{g}"))
            mk.append(mkp.tile([128, MT, GW], dt.uint8, tag="mk", name=f"mk{g}"))

        # DMA priority order (transfers serialize on the shared DMA device):
        # the PE prologue (first f1t half + first f2 half-panel) goes first,
        # then the first tt's mask slice, the rest interleaved by need-time.
        HB = BS // 2
        nc.sync.dma_start(srn1[:], srn1_d[:])
        nc.sync.dma_start(f1t_sb[:, :, 0:HB], f1t_v[:, :, 0:HB])
        nc.sync.dma_start(f2p[0][:, :, 0 : GW // 2], f2nt_v[:, :, 0 : GW // 2])
        nc.sync.dma_start(mk[0][:, 0:2, :], mask_d[:, 0:2, 0:GW])
        nc.sync.dma_start(f2p[0][:, :, GW // 2 : GW], f2nt_v[:, :, GW // 2 : GW])
        nc.sync.dma_start(mk[0][:, 2:MT, :], mask_d[:, 2:MT, 0:GW])
        nc.sync.dma_start(f1t_sb[:, :, HB:BS], f1t_v[:, :, HB:BS])
        nc.sync.dma_start(f2p[1][:], f2nt_v[:, :, GW : 2 * GW])
        nc.sync.dma_start(mk[1][:], mask_d[:, :, GW : 2 * GW])
        for g in range(2, NG):
            nc.sync.dma_start(f2p[g][:], f2nt_v[:, :, g * GW : (g + 1) * GW])
            nc.sync.dma_start(mk[g][:], mask_d[:, :, g * GW : (g + 1) * GW])

        # ---- Main loop: matmul -> mask (DVE tt) -> max (DVE ts) -> exp (ACT)
        def emit_group(g, m, col0, width, stat_col):
            acc = psm.tile([128, width], dt.float32, tag="acc", name="acc")
            for s in range(width // 512):
                for c in range(KC):
                    nc.tensor.matmul(
                        acc[:, s * 512 : (s + 1) * 512],
                        f1t_sb[:, c, m * 128 : (m + 1) * 128],
                        f2p[g][:, c, col0 + s * 512 : col0 + (s + 1) * 512],
                        start=(c == 0),
                        stop=(c == KC - 1),
                    )
            vz = vzp.tile([128, width], dt.bfloat16, tag="vz", name="vz")
            nc.vector.tensor_tensor(
                out=vz[:], in0=acc[:], in1=mk[g][:, m, col0 : col0 + width],
                op=Alu.mult,
            )
            dum = dmp.tile([128, width], dt.bfloat16, tag="dum", name="dum")
            nc.vector.tensor_scalar(
                out=dum[:], in0=vz[:], scalar1=1.0, scalar2=None,
                op0=Alu.mult, op1=Alu.max,
                accum_out=stats[:, stat_col : stat_col + 1],
            )
            ex = exq.tile([128, width], dt.bfloat16, tag="ex", name="ex")
            nc.scalar.activation(
                ex[:],
                vz[:],
                Act.Exp,
                bias=0.0,
                scale=srn1[:, m : m + 1],
                accum_out=sums[:, stat_col : stat_col + 1],
            )

        # extra stats columns NG*MT.. hold the split halves of the last tile
        for g in range(NG):
            for m in range(MT):
                last = g == NG - 1 and m == MT - 1
                if not last:
                    emit_group(g, m, 0, GW, g * MT + m)
                else:
                    # split the final tile so the tail drain is half as long
                    emit_group(g, m, 0, GW // 2, g * MT + m)
                    emit_group(g, m, GW // 2, GW // 2, NG * MT)
            if g < NG - 1:
                # write back this group's stats as soon as its chain drains
                nc.sync.dma_start(mx_d[:, g * MT : (g + 1) * MT],
                                  stats[:, g * MT : (g + 1) * MT])
                nc.sync.dma_start(se_d[:, g * MT : (g + 1) * MT],
                                  sums[:, g * MT : (g + 1) * MT])
        g = NG - 1
        nc.sync.dma_start(mx_d[:, g * MT :], stats[:, g * MT :])
        nc.sync.dma_start(se_d[:, g * MT :], sums[:, g * MT :])

    if not nc.is_finalized():
        nc.finalize()
    return nc


def _get_program():
    if "nc" not in _prog_cache:
        _prog_cache["nc"] = _build_program()
    return _prog_cache["nc"]


def _host_inputs(feature1, feature2, label):
    bf16 = ml_dtypes.bfloat16
    f1 = np.asarray(feature1, dtype=np.float32)
    f2 = np.asarray(feature2, dtype=np.float32)
    lab = np.asarray(label)

    f2n = f2 / np.linalg.norm(f2, axis=1, keepdims=True)
    f2nt = np.ascontiguousarray(f2n.T.astype(bf16))
    rn1_all = 1.0 / np.linalg.norm(f1.astype(np.float64), axis=1)

    in_maps = []
    for c in range(NCORES):
        sl = slice(c * BS, (c + 1) * BS)
        f1s = f1[sl]
        same = lab[sl, None] == lab[None, :]                  # [BS, B]
        maskf = np.ascontiguousarray(
            (~same).astype(np.uint8).reshape(MT, 128, B).transpose(1, 0, 2)
        )
        srn1 = np.ascontiguousarray(
            (S * rn1_all[sl]).reshape(MT, 128).T.astype(np.float32)
        )
        in_maps.append(
            dict(
                f1t=np.ascontiguousarray(f1s.T.astype(bf16)),
                f2nt=f2nt,
                maskf=maskf,
                srn1=srn1,
            )
        )
    return in_maps


def kernel(feature1, feature2, label, _want_results=False, _trace=False):
    f1 = np.asarray(feature1, dtype=np.float32)
    f2 = np.asarray(feature2, dtype=np.float32)
    in_maps = _host_inputs(f1, f2, label)

    nc = _get_program()
    kw = {}
    if _trace:
        kw = dict(trace=True)
    out = run_bass_kernel_spmd(nc, in_maps, list(range(NCORES)), **kw)
    res = out.results

    # host O(B) combine in float64
    f1_64 = f1.astype(np.float64)
    f2_64 = f2.astype(np.float64)
    rn1 = 1.0 / np.linalg.norm(f1_64, axis=1)                 # [B]
    rn2 = 1.0 / np.linalg.norm(f2_64, axis=1)
    pos = np.clip(np.einsum("ij,ij->i", f1_64, f2_64) * rn1 * rn2, -1.0, 1.0)

    neg = np.empty(B, dtype=np.float64)
    sumoff = np.empty(B, dtype=np.float64)
    for c in range(NCORES):
        r = res[c]
        sl = slice(c * BS, (c + 1) * BS)
        mx = r["mx"].astype(np.float64)                       # [128, NG*MT+1]
        se = r["se"].astype(np.float64)
        mxm = mx[:, : NG * MT].reshape(128, NG, MT).max(axis=1)   # [128, MT]
        sem = se[:, : NG * MT].reshape(128, NG, MT).sum(axis=1)
        # fold the split-tile extra column into (g=NG-1, m=MT-1)
        mxm[:, MT - 1] = np.maximum(mxm[:, MT - 1], mx[:, NG * MT])
        sem[:, MT - 1] += se[:, NG * MT]
        neg[sl] = mxm.T.reshape(BS) * rn1[sl]                 # raw-dot max * rn1
        sumoff[sl] = sem.T.reshape(BS) - 1.0

    m = EMA * np.mean(pos - neg)
    z = S * (pos - m)
    loss = np.mean(np.log(sumoff + np.exp(z)) - z)
    out_val = np.float32(loss)
    if _want_results:
        return out_val, out
    return out_val


# revision 15
# speedup vs baseline: 1.9718x; 1.0093x over previous
"""ContraFace loss kernel for 8 TRN2 NeuronCores.

Strategy: row-shard the [B, B] cosine matrix across 8 cores (B/8 = 1024 rows
per core). The device does the O(B^2 D) / O(B^2) work: the full cosine
matmul, same-label masking, per-row hardest-negative max, and per-row
sum(exp(S*cos)). Host supplies layout-prepped shards:
  - f2nt: L2-normalized f2, transposed to [D, B] (bf16) - the matmul moving
    operand, shared by all cores
  - f1t:  raw f1 shard transposed [D, BS] (bf16) - stationary operand
  - maskf: [128, MT, B] uint8 0/1 mask, 0 where label_col == label_row
    (zeroes same-label entries incl. the diagonal, matching the reference's
    cos=0 substitution)
  - srn1: [128, MT] fp32 = S / ||f1_row|| per-partition Exp scale

Device pipeline per core, per (m, g) tile of the [1024, 8192] block:
  - PE: 16 bf16 matmuls accumulate a [128, 2048] PSUM tile (1 cycle/row)
  - DVE tensor_tensor: vz = psum * mask (bf16 out)
  - DVE tensor_scalar on vz (4x bf16 mode) with accum_out op1=max -> row max
  - ACT Exp with per-partition scale srn1 and accum_out -> row sumexp
DMAs are split/ordered so the PE starts ~5.5us in and never starves; the
last m-tile is split in half to shorten the end-of-kernel drain. Host does
the tiny O(B) combine in float64: positives, EMA margin m from (pos - neg),
cross-entropy mean.
"""

import sys
import os

sys.path.insert(0, "/opt/trn_rl_repo")

import numpy as np
from contextlib import ExitStack

import ml_dtypes

from concourse import bass, bacc, tile
from concourse.bass_utils import run_bass_kernel_spmd
import concourse.mybir as mybir

dt = mybir.dt
Alu = mybir.AluOpType
Act = mybir.ActivationFunctionType

B, D = 8192, 512
NCORES = 8
BS = B // NCORES          # 1024 rows per core
MT = BS // 128            # 8 M-tiles per core
KC = D // 128             # 4 contraction chunks
GW = 2048                 # column group width (PSUM tile free size, 4 banks)
NG = B // GW              # 4 column groups
S = 64.0
EMA = 0.99

_prog_cache = {}


def _build_program():
    nc = bacc.Bacc(None)

    f1t_d = nc.declare_dram_parameter("f1t", [D, BS], dt.bfloat16, isOutput=False)
    f2nt_d = nc.declare_dram_parameter("f2nt", [D, B], dt.bfloat16, isOutput=False)
    mask_d = nc.declare_dram_parameter("maskf", [128, MT, B], dt.uint8, isOutput=False)
    srn1_d = nc.declare_dram_parameter("srn1", [128, MT], dt.float32, isOutput=False)

    # g-major stats layout: column g*MT + m, so per-group slices are
    # contiguous and can be written back as soon as each group drains.
    # Extra columns: 32-34 = pieces 2-4 of the split FIRST tile (prologue),
    # 35-36 = pieces 2-3 of the split LAST tile (tail drain).
    NSTAT = NG * MT + 5
    mx_d = nc.declare_dram_parameter("mx", [128, NSTAT], dt.float32, isOutput=True)
    se_d = nc.declare_dram_parameter("se", [128, NSTAT], dt.float32, isOutput=True)

    f1t_v = f1t_d[:].rearrange("(c p) i -> p c i", p=128)
    f2nt_v = f2nt_d[:].rearrange("(c p) j -> p c j", p=128)

    with tile.TileContext(nc) as tc, ExitStack() as ctx:
        cst = ctx.enter_context(tc.tile_pool(name="cst", bufs=1))
        pan = ctx.enter_context(tc.tile_pool(name="pan", bufs=NG))
        mkp = ctx.enter_context(tc.tile_pool(name="mkp", bufs=NG))
        vzp = ctx.enter_context(tc.tile_pool(name="vzp", bufs=4))
        exq = ctx.enter_context(tc.tile_pool(name="exq", bufs=3))
        dmp = ctx.enter_context(tc.tile_pool(name="dmp", bufs=2))
        psm = ctx.enter_context(
            tc.tile_pool(name="psm", bufs=2, space=bass.MemorySpace.PSUM)
        )

        stats = cst.tile([128, NG * MT + 5], dt.float32, tag="stats")
        sums = cst.tile([128, NG * MT + 5], dt.float32, tag="sums")
        srn1 = cst.tile([128, MT], dt.float32, tag="srn1")
        f1t_sb = cst.tile([128, KC, BS], dt.bfloat16, tag="f1t")

        f2p = []
        mk = []
        for g in range(NG):
            f2p.append(pan.tile([128, KC, GW], dt.bfloat16, tag="f2p", name=f"f2p{g}"))
            mk.append(mkp.tile([128, MT, GW], dt.uint8, tag="mk", name=f"mk{g}"))

        # DMA priority order (transfers serialize on the shared DMA device):
        # feed the PE's first 512-wide pieces as fast as possible, then the
        # first tt's mask slice, the rest interleaved by need-time.
        nc.sync.dma_start(srn1[:], srn1_d[:])
        nc.sync.dma_start(f1t_sb[:, :, 0:256], f1t_v[:, :, 0:256])
        nc.sync.dma_start(f2p[0][:, :, 0:512], f2nt_v[:, :, 0:512])
        nc.sync.dma_start(f2p[0][:, :, 512:1024], f2nt_v[:, :, 512:1024])
        nc.sync.dma_start(mk[0][:, 0:2, :], mask_d[:, 0:2, 0:GW])
        nc.sync.dma_start(f2p[0][:, :, 1024:1536], f2nt_v[:, :, 1024:1536])
        nc.sync.dma_start(f2p[0][:, :, 1536:2048], f2nt_v[:, :, 1536:2048])
        nc.sync.dma_start(f1t_sb[:, :, 256:BS], f1t_v[:, :, 256:BS])
        nc.sync.dma_start(mk[0][:, 2:MT, :], mask_d[:, 2:MT, 0:GW])
        nc.sync.dma_start(f2p[1][:], f2nt_v[:, :, GW : 2 * GW])
        nc.sync.dma_start(mk[1][:], mask_d[:, :, GW : 2 * GW])
        for g in range(2, NG):
            nc.sync.dma_start(f2p[g][:], f2nt_v[:, :, g * GW : (g + 1) * GW])
            nc.sync.dma_start(mk[g][:], mask_d[:, :, g * GW : (g + 1) * GW])

        # ---- Main loop: matmul -> mask (DVE tt) -> max (DVE ts) -> exp (ACT)
        def emit_group(g, m, col0, width, stat_col):
            acc = psm.tile([128, width], dt.float32, tag="acc", name="acc")
            for s in range(width // 512):
                for c in range(KC):
                    nc.tensor.matmul(
                        acc[:, s * 512 : (s + 1) * 512],
                        f1t_sb[:, c, m * 128 : (m + 1) * 128],
                        f2p[g][:, c, col0 + s * 512 : col0 + (s + 1) * 512],
                        start=(c == 0),
                        stop=(c == KC - 1),
                    )
            vz = vzp.tile([128, width], dt.bfloat16, tag="vz", name="vz")
            nc.vector.tensor_tensor(
                out=vz[:], in0=acc[:], in1=mk[g][:, m, col0 : col0 + width],
                op=Alu.mult,
            )
            dum = dmp.tile([128, width], dt.bfloat16, tag="dum", name="dum")
            nc.vector.tensor_scalar(
                out=dum[:], in0=vz[:], scalar1=1.0, scalar2=None,
                op0=Alu.mult, op1=Alu.max,
                accum_out=stats[:, stat_col : stat_col + 1],
            )
            ex = exq.tile([128, width], dt.bfloat16, tag="ex", name="ex")
            nc.scalar.activation(
                ex[:],
                vz[:],
                Act.Exp,
                bias=0.0,
                scale=srn1[:, m : m + 1],
                accum_out=sums[:, stat_col : stat_col + 1],
            )

        for g in range(NG):
            for m in range(MT):
                first = g == 0 and m == 0
                last = g == NG - 1 and m == MT - 1
                if first:
                    # 4 x 512 pieces: PE starts as soon as the first 512
                    # columns of f2p[0] land
                    emit_group(g, m, 0, 512, 0)
                    for piece in range(1, 4):
                        emit_group(g, m, piece * 512, 512, NG * MT - 1 + piece)
                elif last:
                    # 1024 + 512 + 512 so the tail drain chain is short
                    emit_group(g, m, 0, GW // 2, g * MT + m)
                    emit_group(g, m, GW // 2, 512, NG * MT + 3)
                    emit_group(g, m, GW // 2 + 512, 512, NG * MT + 4)
                else:
                    emit_group(g, m, 0, GW, g * MT + m)
            if g < NG - 1:
                # write back this group's stats as soon as its chain drains
                nc.sync.dma_start(mx_d[:, g * MT : (g + 1) * MT],
                                  stats[:, g * MT : (g + 1) * MT])
                nc.sync.dma_start(se_d[:, g * MT : (g + 1) * MT],
                                  sums[:, g * MT : (g + 1) * MT])
        # final writeback: last group's columns plus all extra split columns
        g = NG - 1
        nc.sync.dma_start(mx_d[:, g * MT :], stats[:, g * MT :])
        nc.sync.dma_start(se_d[:, g * MT :], sums[:, g * MT :])

    if not nc.is_finalized():
        nc.finalize()
    return nc


def _get_program():
    if "nc" not in _prog_cache:
        _prog_cache["nc"] = _build_program()
    return _prog_cache["nc"]


def _host_inputs(feature1, feature2, label):
    bf16 = ml_dtypes.bfloat16
    f1 = np.asarray(feature1, dtype=np.float32)
    f2 = np.asarray(feature2, dtype=np.float32)
    lab = np.asarray(label)

    f2n = f2 / np.linalg.norm(f2, axis=1, keepdims=True)
    f2nt = np.ascontiguousarray(f2n.T.astype(bf16))
    rn1_all = 1.0 / np.linalg.norm(f1.astype(np.float64), axis=1)

    in_maps = []
    for c in range(NCORES):
        sl = slice(c * BS, (c + 1) * BS)
        f1s = f1[sl]
        same = lab[sl, None] == lab[None, :]                  # [BS, B]
        maskf = np.ascontiguousarray(
            (~same).astype(np.uint8).reshape(MT, 128, B).transpose(1, 0, 2)
        )
        srn1 = np.ascontiguousarray(
            (S * rn1_all[sl]).reshape(MT, 128).T.astype(np.float32)
        )
        in_maps.append(
            dict(
                f1t=np.ascontiguousarray(f1s.T.astype(bf16)),
                f2nt=f2nt,
                maskf=maskf,
                srn1=srn1,
            )
        )
    return in_maps


def kernel(feature1, feature2, label, _want_results=False, _trace=False):
    f1 = np.asarray(feature1, dtype=np.float32)
    f2 = np.asarray(feature2, dtype=np.float32)
    in_maps = _host_inputs(f1, f2, label)

    nc = _get_program()
    kw = {}
    if _trace:
        kw = dict(trace=True)
    out = run_bass_kernel_spmd(nc, in_maps, list(range(NCORES)), **kw)
    res = out.results

    # host O(B) combine in float64
    f1_64 = f1.astype(np.float64)
    f2_64 = f2.astype(np.float64)
    rn1 = 1.0 / np.linalg.norm(f1_64, axis=1)                 # [B]
    rn2 = 1.0 / np.linalg.norm(f2_64, axis=1)
    pos = np.clip(np.einsum("ij,ij->i", f1_64, f2_64) * rn1 * rn2, -1.0, 1.0)

    neg = np.empty(B, dtype=np.float64)
    sumoff = np.empty(B, dtype=np.float64)
    for c in range(NCORES):
        r = res[c]
        sl = slice(c * BS, (c + 1) * BS)
        mx = r["mx"].astype(np.float64)                       # [128, NG*MT+5]
        se = r["se"].astype(np.float64)
        mxm = mx[:, : NG * MT].reshape(128, NG, MT).max(axis=1)   # [128, MT]
        sem = se[:, : NG * MT].reshape(128, NG, MT).sum(axis=1)
        # fold split-tile extras: cols 32-34 -> (g0, m0); 35-36 -> (g3, m7)
        mxm[:, 0] = np.maximum(mxm[:, 0], mx[:, NG * MT : NG * MT + 3].max(axis=1))
        sem[:, 0] += se[:, NG * MT : NG * MT + 3].sum(axis=1)
        mxm[:, MT - 1] = np.maximum(mxm[:, MT - 1], mx[:, NG * MT + 3 :].max(axis=1))
        sem[:, MT - 1] += se[:, NG * MT + 3 :].sum(axis=1)
        neg[sl] = mxm.T.reshape(BS) * rn1[sl]                 # raw-dot max * rn1
        sumoff[sl] = sem.T.reshape(BS) - 1.0

    m = EMA * np.mean(pos - neg)
    z = S * (pos - m)
    loss = np.mean(np.log(sumoff + np.exp(z)) - z)
    out_val = np.float32(loss)
    if _want_results:
        return out_val, out
    return out_val


# revision 20
# speedup vs baseline: 1.9792x; 1.0038x over previous
"""ContraFace loss kernel for 8 TRN2 NeuronCores.

Strategy: row-shard the [B, B] cosine matrix across 8 cores (B/8 = 1024 rows
per core). The device does the O(B^2 D) / O(B^2) work: the full cosine
matmul, same-label masking, per-row hardest-negative max, and per-row
sum(exp(S*cos)). Host supplies layout-prepped shards:
  - f2nt: L2-normalized f2, transposed to [D, B] (bf16) - the matmul moving
    operand, shared by all cores
  - f1t:  raw f1 shard transposed [D, BS] (bf16) - stationary operand
  - maskf: [128, MT, B] uint8 0/1 mask, 0 where label_col == label_row
    (zeroes same-label entries incl. the diagonal, matching the reference's
    cos=0 substitution)
  - srn1: [128, MT] fp32 = S / ||f1_row|| per-partition Exp scale

Device pipeline per core, per (m, g) tile of the [1024, 8192] block:
  - PE: 16 bf16 matmuls accumulate a [128, 2048] PSUM tile (1 cycle/row)
  - DVE tensor_tensor: vz = psum * mask (bf16 out)
  - DVE tensor_scalar on vz (4x bf16 mode) with accum_out op1=max -> row max
  - ACT Exp with per-partition scale srn1 and accum_out -> row sumexp
DMAs are split/ordered so the PE starts ~5.5us in and never starves; the
last m-tile is split in half to shorten the end-of-kernel drain. Host does
the tiny O(B) combine in float64: positives, EMA margin m from (pos - neg),
cross-entropy mean.
"""

import sys
import os

sys.path.insert(0, "/opt/trn_rl_repo")

import numpy as np
from contextlib import ExitStack

import ml_dtypes

from concourse import bass, bacc, tile
from concourse.bass_utils import run_bass_kernel_spmd
import concourse.mybir as mybir

dt = mybir.dt
Alu = mybir.AluOpType
Act = mybir.ActivationFunctionType

B, D = 8192, 512
NCORES = 8
BS = B // NCORES          # 1024 rows per core
MT = BS // 128            # 8 M-tiles per core
KC = D // 128             # 4 contraction chunks
GW = 2048                 # column group width (PSUM tile free size, 4 banks)
NG = B // GW              # 4 column groups
S = 64.0
EMA = 0.99

_prog_cache = {}


def _build_program():
    nc = bacc.Bacc(None)

    f1t_d = nc.declare_dram_parameter("f1t", [D, BS], dt.bfloat16, isOutput=False)
    f2nt_d = nc.declare_dram_parameter("f2nt", [D, B], dt.bfloat16, isOutput=False)
    mask_d = nc.declare_dram_parameter("maskf", [128, MT, B], dt.uint8, isOutput=False)
    srn1_d = nc.declare_dram_parameter("srn1", [128, MT], dt.float32, isOutput=False)

    # g-major stats layout: column g*MT + m, so per-group slices are
    # contiguous and can be written back as soon as each group drains.
    # Extra columns: 32-34 = pieces 2-4 of split tile (g0,m0), 35-37 =
    # pieces 2-4 of split tile (g0,m1) [prologue DMA race], 38-39 =
    # pieces 2-3 of the split LAST tile (tail drain).
    NSTAT = NG * MT + 8
    mx_d = nc.declare_dram_parameter("mx", [128, NSTAT], dt.float32, isOutput=True)
    se_d = nc.declare_dram_parameter("se", [128, NSTAT], dt.float32, isOutput=True)

    f1t_v = f1t_d[:].rearrange("(c p) i -> p c i", p=128)
    f2nt_v = f2nt_d[:].rearrange("(c p) j -> p c j", p=128)

    with tile.TileContext(nc) as tc, ExitStack() as ctx:
        cst = ctx.enter_context(tc.tile_pool(name="cst", bufs=1))
        pan = ctx.enter_context(tc.tile_pool(name="pan", bufs=NG))
        mkp = ctx.enter_context(tc.tile_pool(name="mkp", bufs=NG))
        vzp = ctx.enter_context(tc.tile_pool(name="vzp", bufs=4))
        exq = ctx.enter_context(tc.tile_pool(name="exq", bufs=3))
        dmp = ctx.enter_context(tc.tile_pool(name="dmp", bufs=2))
        psm = ctx.enter_context(
            tc.tile_pool(name="psm", bufs=2, space=bass.MemorySpace.PSUM)
        )

        stats = cst.tile([128, NG * MT + 8], dt.float32, tag="stats")
        sums = cst.tile([128, NG * MT + 8], dt.float32, tag="sums")
        srn1 = cst.tile([128, MT], dt.float32, tag="srn1")
        f1t_sb = cst.tile([128, KC, BS], dt.bfloat16, tag="f1t")

        f2p = []
        mk = []
        for g in range(NG):
            f2p.append(pan.tile([128, KC, GW], dt.bfloat16, tag="f2p", name=f"f2p{g}"))
            mk.append(mkp.tile([128, MT, GW], dt.uint8, tag="mk", name=f"mk{g}"))

        # DMA priority order (transfers serialize on the shared DMA device):
        # feed the PE's first 512-wide pieces as fast as possible (m0 and m1
        # interleave on each quarter so the PE matches the DMA rate), each
        # quarter's mask slice right behind it, the rest by need-time.
        nc.sync.dma_start(srn1[:], srn1_d[:])
        nc.sync.dma_start(f1t_sb[:, :, 0:256], f1t_v[:, :, 0:256])
        nc.sync.dma_start(f2p[0][:, :, 0:512], f2nt_v[:, :, 0:512])
        nc.sync.dma_start(mk[0][:, 0:2, 0:512], mask_d[:, 0:2, 0:512])
        nc.sync.dma_start(f2p[0][:, :, 512:1024], f2nt_v[:, :, 512:1024])
        nc.sync.dma_start(f1t_sb[:, :, 256:512], f1t_v[:, :, 256:512])
        nc.sync.dma_start(mk[0][:, 0:2, 512:1024], mask_d[:, 0:2, 512:1024])
        nc.sync.dma_start(f2p[0][:, :, 1024:1536], f2nt_v[:, :, 1024:1536])
        nc.sync.dma_start(mk[0][:, 0:2, 1024:1536], mask_d[:, 0:2, 1024:1536])
        nc.sync.dma_start(f2p[0][:, :, 1536:2048], f2nt_v[:, :, 1536:2048])
        nc.sync.dma_start(mk[0][:, 0:2, 1536:2048], mask_d[:, 0:2, 1536:2048])
        nc.sync.dma_start(f1t_sb[:, :, 512:BS], f1t_v[:, :, 512:BS])
        nc.sync.dma_start(mk[0][:, 2:MT, :], mask_d[:, 2:MT, 0:GW])
        nc.sync.dma_start(f2p[1][:], f2nt_v[:, :, GW : 2 * GW])
        nc.sync.dma_start(mk[1][:], mask_d[:, :, GW : 2 * GW])
        for g in range(2, NG):
            nc.sync.dma_start(f2p[g][:], f2nt_v[:, :, g * GW : (g + 1) * GW])
            nc.sync.dma_start(mk[g][:], mask_d[:, :, g * GW : (g + 1) * GW])

        # ---- Main loop: matmul -> mask (DVE tt) -> max (DVE ts) -> exp (ACT)
        def emit_group(g, m, col0, width, stat_col):
            acc = psm.tile([128, width], dt.float32, tag="acc", name="acc")
            for s in range(width // 512):
                for c in range(KC):
                    nc.tensor.matmul(
                        acc[:, s * 512 : (s + 1) * 512],
                        f1t_sb[:, c, m * 128 : (m + 1) * 128],
                        f2p[g][:, c, col0 + s * 512 : col0 + (s + 1) * 512],
                        start=(c == 0),
                        stop=(c == KC - 1),
                    )
            vz = vzp.tile([128, width], dt.bfloat16, tag="vz", name="vz")
            nc.vector.tensor_tensor(
                out=vz[:], in0=acc[:], in1=mk[g][:, m, col0 : col0 + width],
                op=Alu.mult,
            )
            dum = dmp.tile([128, width], dt.bfloat16, tag="dum", name="dum")
            nc.vector.tensor_scalar(
                out=dum[:], in0=vz[:], scalar1=1.0, scalar2=None,
                op0=Alu.mult, op1=Alu.max,
                accum_out=stats[:, stat_col : stat_col + 1],
            )
            ex = exq.tile([128, width], dt.bfloat16, tag="ex", name="ex")
            nc.scalar.activation(
                ex[:],
                vz[:],
                Act.Exp,
                bias=0.0,
                scale=srn1[:, m : m + 1],
                accum_out=sums[:, stat_col : stat_col + 1],
            )

        # prologue: m0/m1 of g0 in interleaved 512-wide pieces, pacing the PE
        # to the DMA arrival rate of the f2p[0] quarters
        for piece in range(4):
            for m in range(2):
                col = m if piece == 0 else NG * MT + m * 3 + (piece - 1)
                emit_group(0, m, piece * 512, 512, col)
        for g in range(NG):
            for m in range(MT):
                if g == 0 and m < 2:
                    continue
                last = g == NG - 1 and m == MT - 1
                if last:
                    # 1024 + 512 + 512 so the tail drain chain is short
                    emit_group(g, m, 0, GW // 2, g * MT + m)
                    emit_group(g, m, GW // 2, 512, NG * MT + 6)
                    emit_group(g, m, GW // 2 + 512, 512, NG * MT + 7)
                else:
                    emit_group(g, m, 0, GW, g * MT + m)
            if g < NG - 1:
                # write back this group's stats as soon as its chain drains
                nc.sync.dma_start(mx_d[:, g * MT : (g + 1) * MT],
                                  stats[:, g * MT : (g + 1) * MT])
                nc.sync.dma_start(se_d[:, g * MT : (g + 1) * MT],
                                  sums[:, g * MT : (g + 1) * MT])
        # final writeback: last group's columns plus all extra split columns
        g = NG - 1
        nc.sync.dma_start(mx_d[:, g * MT :], stats[:, g * MT :])
        nc.sync.dma_start(se_d[:, g * MT :], sums[:, g * MT :])

    if not nc.is_finalized():
        nc.finalize()
    return nc


def _get_program():
    if "nc" not in _prog_cache:
        _prog_cache["nc"] = _build_program()
    return _prog_cache["nc"]


def _host_inputs(feature1, feature2, label):
    bf16 = ml_dtypes.bfloat16
    f1 = np.asarray(feature1, dtype=np.float32)
    f2 = np.asarray(feature2, dtype=np.float32)
    lab = np.asarray(label)

    f2n = f2 / np.linalg.norm(f2, axis=1, keepdims=True)
    f2nt = np.ascontiguousarray(f2n.T.astype(bf16))
    rn1_all = 1.0 / np.linalg.norm(f1.astype(np.float64), axis=1)

    in_maps = []
    for c in range(NCORES):
        sl = slice(c * BS, (c + 1) * BS)
        f1s = f1[sl]
        same = lab[sl, None] == lab[None, :]                  # [BS, B]
        maskf = np.ascontiguousarray(
            (~same).astype(np.uint8).reshape(MT, 128, B).transpose(1, 0, 2)
        )
        srn1 = np.ascontiguousarray(
            (S * rn1_all[sl]).reshape(MT, 128).T.astype(np.float32)
        )
        in_maps.append(
            dict(
                f1t=np.ascontiguousarray(f1s.T.astype(bf16)),
                f2nt=f2nt,
                maskf=maskf,
                srn1=srn1,
            )
        )
    return in_maps


def kernel(feature1, feature2, label, _want_results=False, _trace=False):
    f1 = np.asarray(feature1, dtype=np.float32)
    f2 = np.asarray(feature2, dtype=np.float32)
    in_maps = _host_inputs(f1, f2, label)

    nc = _get_program()
    kw = {}
    if _trace:
        kw = dict(trace=True)
    out = run_bass_kernel_spmd(nc, in_maps, list(range(NCORES)), **kw)
    res = out.results

    # host O(B) combine in float64
    f1_64 = f1.astype(np.float64)
    f2_64 = f2.astype(np.float64)
    rn1 = 1.0 / np.linalg.norm(f1_64, axis=1)                 # [B]
    rn2 = 1.0 / np.linalg.norm(f2_64, axis=1)
    pos = np.clip(np.einsum("ij,ij->i", f1_64, f2_64) * rn1 * rn2, -1.0, 1.0)

    neg = np.empty(B, dtype=np.float64)
    sumoff = np.empty(B, dtype=np.float64)
    for c in range(NCORES):
        r = res[c]
        sl = slice(c * BS, (c + 1) * BS)
        mx = r["mx"].astype(np.float64)                       # [128, NG*MT+5]
        se = r["se"].astype(np.float64)
        mxm = mx[:, : NG * MT].reshape(128, NG, MT).max(axis=1)   # [128, MT]
        sem = se[:, : NG * MT].reshape(128, NG, MT).sum(axis=1)
        # fold split-tile extras: 32-34 -> (g0,m0); 35-37 -> (g0,m1);
        # 38-39 -> (g3,m7)
        E = NG * MT
        mxm[:, 0] = np.maximum(mxm[:, 0], mx[:, E : E + 3].max(axis=1))
        sem[:, 0] += se[:, E : E + 3].sum(axis=1)
        mxm[:, 1] = np.maximum(mxm[:, 1], mx[:, E + 3 : E + 6].max(axis=1))
        sem[:, 1] += se[:, E + 3 : E + 6].sum(axis=1)
        mxm[:, MT - 1] = np.maximum(mxm[:, MT - 1], mx[:, E + 6 :].max(axis=1))
        sem[:, MT - 1] += se[:, E + 6 :].sum(axis=1)
        neg[sl] = mxm.T.reshape(BS) * rn1[sl]                 # raw-dot max * rn1
        sumoff[sl] = sem.T.reshape(BS) - 1.0

    m = EMA * np.mean(pos - neg)
    z = S * (pos - m)
    loss = np.mean(np.log(sumoff + np.exp(z)) - z)
    out_val = np.float32(loss)
    if _want_results:
        return out_val, out
    return out_val
